# revision 1
# baseline (speedup 1.0000x reference)
"""Trainium2 Bass kernel for EntropyAndMutualInformation.

Full inputs act_X, act_Y [8192, 4096] f32. Output [2] f32: [entropy, mi].

Strategy (8 NeuronCores):
  Phase 1 (data-parallel over rows): each core softmaxes its 1024-row shard
    of X and Y (no max-subtraction needed: |x| < 6), emits bf16 probs plus
    per-row Z and dot(e, x) for the entropy (finished on host in fp64).
  Phase 2: AllGather of probs_X (bf16) -> every core holds full probs_X;
    AllToAll of probs_Y column-blocks -> core k holds probs_Y[:, kW:(k+1)W]
    over all 8192 rows (W = 512).
  Phase 3: each core computes its W-column block of the unnormalized joint
    T = probs_X^T @ probs_Y_block over the full contraction (K=8192) in
    128x512 PSUM tiles, immediately consumed: ln(T + N*eps) on ScalarE,
    fused multiply-reduce on VectorE into sum(T*ln(T+N*eps)) accumulators,
    and per-m-tile row-sums of T (which are exactly the X-marginal of the
    computed joint). The Y-marginal block comes from ones-matmuls over the
    resident rhs.
  Host (fp64): entropy = mean(ln Z - dot/Z);
    S1 = (sum T ln(T+N eps) - ln(N) sum T)/N; since softmax rows sum to 1,
    row/col sums of the joint are the marginals, so
    MI = S1 - sum mX ln(mX+eps) - sum mY ln(mY+eps).

DMA engine plan (avoids HWDGE FIFO head-of-line blocking):
  phase-1 input loads -> scalar ring; probs/dx stores -> vector ring;
  zx stores -> scalar ring; collectives -> gpsimd; phase-3 slab/rhs
  streams -> sync ring.
"""
import sys

sys.path.insert(0, "/opt/trn_rl_repo")

import numpy as np

N_TOTAL, C_DIM, N_CORES = 8192, 4096, 8
EPS = 1e-12


def build_nc(n_total=N_TOTAL, C=C_DIM, ncores=N_CORES, m_group=4, k_chunk=32,
             debug=False, colls="both", stage=4, dma_split=True, halves=True):
    if halves:
        return build_nc_halves(n_total, C, ncores, m_group, k_chunk, debug,
                               colls, stage, dma_split)
    return build_nc_single(n_total, C, ncores, m_group, k_chunk, debug,
                           colls, stage, dma_split)


def build_nc_single(n_total=N_TOTAL, C=C_DIM, ncores=N_CORES, m_group=4,
                    k_chunk=32, debug=False, colls="both", stage=4,
                    dma_split=True):
    import concourse.bass as bass
    import concourse.tile as tile
    import concourse.mybir as mybir
    from concourse import bacc

    f32 = mybir.dt.float32
    bf16 = mybir.dt.bfloat16
    P = 128
    n_shard = n_total // ncores
    W = C // ncores              # rhs / joint column-block width per core
    assert W <= 512
    row_tiles = n_shard // P
    k_tiles = n_total // P
    m_tiles = C // P
    k_chunk = min(k_chunk, k_tiles)
    assert k_tiles % k_chunk == 0
    mg = min(m_group, m_tiles)
    assert m_tiles % mg == 0
    n_groups = m_tiles // mg

    nc = bacc.Bacc("TRN2", target_bir_lowering=False, debug=debug,
                   enable_asserts=True, num_devices=ncores)

    x_in = nc.dram_tensor("x", [n_shard, C], f32, kind="ExternalInput").ap()
    y_in = nc.dram_tensor("y", [n_shard, C], f32, kind="ExternalInput").ap()

    zx_out = nc.dram_tensor("zx", [row_tiles, P, 1], f32, kind="ExternalOutput").ap()
    dx_out = nc.dram_tensor("dx", [row_tiles, P, 1], f32, kind="ExternalOutput").ap()
    # per-m-tile row sums of T: margxr[m, p] = sum_d T[m*128+p, d_block]
    margxr_out = nc.dram_tensor("margxr", [m_tiles, P, 1], f32,
                                kind="ExternalOutput").ap()
    margy_out = nc.dram_tensor("margy", [1, W], f32, kind="ExternalOutput").ap()
    tlogt_out = nc.dram_tensor("tlogt", [P, 1], f32, kind="ExternalOutput").ap()

    # Collective buffers (DRAM)
    agx_in = nc.dram_tensor("agx_in", [n_shard, C], bf16)
    agx_out = nc.dram_tensor("agx_out", [n_total, C], bf16, addr_space="Shared")
    a2a_in = nc.dram_tensor("a2a_in", [ncores, n_shard, W], bf16)
    a2a_out = nc.dram_tensor("a2a_out", [ncores, n_shard, W], bf16)

    Exp = mybir.ActivationFunctionType.Exp
    Ln = mybir.ActivationFunctionType.Ln
    mult = mybir.AluOpType.mult
    add = mybir.AluOpType.add

    if dma_split:
        eng_in, eng_out, eng_tiny = nc.scalar, nc.sync, nc.scalar
    else:
        eng_in = eng_out = eng_tiny = nc.sync

    # strided view of a2a_in for the single-DMA scatter write:
    # [t][p, j, w] <- probs_Y tile [p, (j w)]
    a2a_in_v = a2a_in[:].rearrange("j (t p) w -> t p j w", p=P)

    with tile.TileContext(nc) as tc:
        # ---------------- Phase 1: local softmax ----------------
        with (
            tc.tile_pool(name="pin", bufs=4) as pin,
            tc.tile_pool(name="pe_", bufs=3) as pe_,
            tc.tile_pool(name="ppr", bufs=3) as ppr,
            tc.tile_pool(name="pscr", bufs=2) as pscr,
            tc.tile_pool(name="p1s", bufs=8) as p1s,
        ):
            for t in range(row_tiles if stage != 5 else 0):
                # ---- Y first (feeds the A2A) ----
                yt = pin.tile([P, C], f32, tag="xt")
                eng_in.dma_start(yt[:], y_in[t * P:(t + 1) * P, :])
                ey = pe_.tile([P, C], f32, tag="et")
                zy = p1s.tile([P, 1], f32, tag="z")
                nc.scalar.activation(ey[:], yt[:], Exp, accum_out=zy[:])
                rzy = p1s.tile([P, 1], f32, tag="rz")
                nc.vector.reciprocal(rzy[:], zy[:])
                pyt = ppr.tile([P, C], bf16, tag="pt")
                nc.vector.tensor_scalar_mul(pyt[:], ey[:], rzy[:])
                eng_out.dma_start(
                    a2a_in_v[t], pyt[:].rearrange("p (j w) -> p j w", j=ncores))

                # ---- X ----
                xt = pin.tile([P, C], f32, tag="xt")
                eng_in.dma_start(xt[:], x_in[t * P:(t + 1) * P, :])
                ex = pe_.tile([P, C], f32, tag="et")
                zx = p1s.tile([P, 1], f32, tag="z")
                nc.scalar.activation(ex[:], xt[:], Exp, accum_out=zx[:])
                eng_tiny.dma_start(zx_out[t], zx[:])
                rzx = p1s.tile([P, 1], f32, tag="rz")
                nc.vector.reciprocal(rzx[:], zx[:])
                pxt = ppr.tile([P, C], bf16, tag="pt")
                nc.vector.tensor_scalar_mul(pxt[:], ex[:], rzx[:])
                eng_out.dma_start(agx_in[t * P:(t + 1) * P, :], pxt[:])
                # entropy: dot(e, x) per row
                scr = pscr.tile([P, C], f32, tag="scr")
                dx = p1s.tile([P, 1], f32, tag="dx")
                nc.vector.scalar_tensor_tensor(
                    out=scr[:], in0=ex[:], scalar=1.0, in1=xt[:],
                    op0=mult, op1=mult, accum_out=dx[:])
                eng_out.dma_start(dx_out[t], dx[:])

            # ---------------- Phase 2: collectives ----------------
            rg = [list(range(ncores))]
            if stage < 2 or stage == 5:
                rg = None  # skip collectives
            if rg is not None and colls in ("both", "a2a"):
                nc.gpsimd.collective_compute(
                    "AllToAll", mybir.AluOpType.bypass, replica_groups=rg,
                    ins=[a2a_in[:]], outs=[a2a_out[:]])
            elif rg is not None:
                for j in range(ncores):
                    nc.sync.dma_start(a2a_out[j], a2a_in[j])
            if rg is not None and colls in ("both", "ag"):
                nc.gpsimd.collective_compute(
                    "AllGather", mybir.AluOpType.bypass, replica_groups=rg,
                    ins=[agx_in[:]], outs=[agx_out[:]])
            elif rg is not None:
                for r in range(ncores):
                    nc.sync.dma_start(
                        agx_out[r * n_shard:(r + 1) * n_shard, :], agx_in[:])

        # ---------------- Phase 3: joint block matmul ----------------
        if stage < 3:
            pass
        else:
          rhs_view = a2a_out[:].rearrange("j (kk p) w -> p (j kk) w", p=P)
          lhs_view = agx_out[:].rearrange("(k p) c -> p k c", p=P)
          with (
            tc.tile_pool(name="rhsp", bufs=1) as rhsp,
            tc.tile_pool(name="constp3", bufs=1) as constp3,
          ):
            rhs = rhsp.tile([P, k_tiles, W], bf16)
            rhs_chunk = max(1, 4096 // W)
            for kc in range(0, k_tiles, rhs_chunk):
                ke = min(k_tiles, kc + rhs_chunk)
                nc.sync.dma_start(rhs[:, kc:ke, :], rhs_view[:, kc:ke, :])

            ones3 = constp3.tile([P, 1], bf16)
            nc.vector.memset(ones3[:], 1.0)
            ln_bias = constp3.tile([P, 1], f32)
            nc.vector.memset(ln_bias[:], float(n_total) * EPS)

            if stage >= 4:
              with (
                tc.tile_pool(name="slabp", bufs=3) as slabp,
                tc.tile_pool(name="jpsum", bufs=2 * mg, space="PSUM") as jpsum,
                tc.tile_pool(name="drain", bufs=3) as drainp,
                tc.tile_pool(name="accp", bufs=4) as accp,
              ):
                acc_t_prev = None
                for g in range(n_groups):
                    psums = [jpsum.tile([P, W], f32, tag="jp", name=f"jp_{g}_{m}")
                             for m in range(mg)]
                    for kc in range(0, k_tiles, k_chunk):
                        slab = slabp.tile([P, k_chunk, mg * P], bf16, tag="slab")
                        nc.sync.dma_start(
                            slab[:],
                            lhs_view[:, kc:kc + k_chunk,
                                     g * mg * P:(g + 1) * mg * P])
                        for kk in range(k_chunk):
                            k = kc + kk
                            for m in range(mg):
                                nc.tensor.matmul(
                                    psums[m][:],
                                    slab[:, kk, m * P:(m + 1) * P],
                                    rhs[:, k, :],
                                    start=(k == 0), stop=(k == k_tiles - 1))
                    for m in range(mg):
                        lnt = drainp.tile([P, W], f32, tag="lnt")
                        nc.scalar.activation(lnt[:], psums[m][:], Ln,
                                             bias=ln_bias[:])
                        scr1 = drainp.tile([P, W], f32, tag="scr1")
                        tt_tmp = accp.tile([P, 1], f32, tag="tttmp")
                        nc.vector.scalar_tensor_tensor(
                            out=scr1[:], in0=psums[m][:], scalar=1.0,
                            in1=lnt[:], op0=mult, op1=mult,
                            accum_out=tt_tmp[:])
                        acc_t = accp.tile([P, 1], f32, tag="acct")
                        if acc_t_prev is None:
                            nc.vector.tensor_copy(acc_t[:], tt_tmp[:])
                        else:
                            nc.vector.tensor_add(acc_t[:], acc_t_prev[:],
                                                 tt_tmp[:])
                        acc_t_prev = acc_t
                        # row sums of T for this m-tile (X-marginal piece)
                        scr2 = drainp.tile([P, W], f32, tag="scr2")
                        st_m = accp.tile([P, 1], f32, tag="stm")
                        nc.vector.tensor_scalar(
                            out=scr2[:], in0=psums[m][:], scalar1=1.0,
                            scalar2=None, op0=mult, op1=add,
                            accum_out=st_m[:])
                        eng_out.dma_start(margxr_out[g * mg + m], st_m[:])

                eng_out.dma_start(tlogt_out[:], acc_t_prev[:])

                # margY for this core's column block (after the joint loop)
                psum_my = jpsum.tile([1, W], f32, tag="jp", name="psum_my")
                for k in range(k_tiles):
                    nc.tensor.matmul(psum_my[:], ones3[:, 0:1], rhs[:, k, :],
                                     start=(k == 0), stop=(k == k_tiles - 1))
                margy_sb = constp3.tile([1, W], f32)
                nc.vector.tensor_copy(margy_sb[:], psum_my[:])
                eng_out.dma_start(margy_out[:], margy_sb[:])

    nc.compile()
    return nc


def build_nc_halves(n_total=N_TOTAL, C=C_DIM, ncores=N_CORES, m_group=4,
                    k_chunk=16, debug=False, colls="both", stage=4,
                    dma_split=True):
    """Half-split pipeline: collectives fire per row-half so the first half's
    AllToAll/AllGather complete during phase 1; the first pair of m-groups
    runs its half-0 contraction inside the exposed window."""
    import concourse.bass as bass
    import concourse.tile as tile
    import concourse.mybir as mybir
    from concourse import bacc

    f32 = mybir.dt.float32
    bf16 = mybir.dt.bfloat16
    P = 128
    n_shard = n_total // ncores
    W = C // ncores
    assert W <= 512
    row_tiles = n_shard // P
    assert row_tiles % 2 == 0
    ht = row_tiles // 2            # phase-1 tiles per half
    hrows = n_shard // 2           # rows per half per rank
    k_tiles = n_total // P
    kh = k_tiles // 2              # k-tiles per half
    m_tiles = C // P
    mg = min(m_group, m_tiles)
    assert m_tiles % mg == 0
    n_groups = m_tiles // mg
    k_chunk = min(k_chunk, kh)
    assert kh % k_chunk == 0

    nc = bacc.Bacc("TRN2", target_bir_lowering=False, debug=debug,
                   enable_asserts=True, num_devices=ncores)

    x_in = nc.dram_tensor("x", [n_shard, C], f32, kind="ExternalInput").ap()
    y_in = nc.dram_tensor("y", [n_shard, C], f32, kind="ExternalInput").ap()

    zx_out = nc.dram_tensor("zx", [row_tiles, P, 1], f32, kind="ExternalOutput").ap()
    dx_out = nc.dram_tensor("dx", [row_tiles, P, 1], f32, kind="ExternalOutput").ap()
    margxr_out = nc.dram_tensor("margxr", [m_tiles, P, 1], f32,
                                kind="ExternalOutput").ap()
    margy_out = nc.dram_tensor("margy", [1, W], f32, kind="ExternalOutput").ap()
    tlogt_out = nc.dram_tensor("tlogt", [P, 1], f32, kind="ExternalOutput").ap()

    agx_in = nc.dram_tensor("agx_in", [n_shard, C], bf16)
    agx_out_h = [nc.dram_tensor(f"agx_out_h{h}", [ncores * hrows, C], bf16,
                                addr_space="Shared") for h in range(2)]
    a2a_in_h = [nc.dram_tensor(f"a2a_in_h{h}", [ncores, hrows, W], bf16)
                for h in range(2)]
    a2a_out_h = [nc.dram_tensor(f"a2a_out_h{h}", [ncores, hrows, W], bf16)
                 for h in range(2)]

    Exp = mybir.ActivationFunctionType.Exp
    Ln = mybir.ActivationFunctionType.Ln
    mult = mybir.AluOpType.mult
    add = mybir.AluOpType.add

    if dma_split:
        eng_in, eng_out, eng_tiny = nc.scalar, nc.sync, nc.scalar
    else:
        eng_in = eng_out = eng_tiny = nc.sync

    # per-half scatter views: [t_local][p, j, w]
    a2a_in_v = [a2a_in_h[h][:].rearrange("j (t p) w -> t p j w", p=P)
                for h in range(2)]
    rg = [list(range(ncores))]

    def emit_collectives(h):
        if stage < 2 or stage == 5:
            do_coll = None
        else:
            do_coll = colls
        if do_coll in ("both", "a2a"):
            nc.gpsimd.collective_compute(
                "AllToAll", mybir.AluOpType.bypass, replica_groups=rg,
                ins=[a2a_in_h[h][:]], outs=[a2a_out_h[h][:]])
        elif do_coll is not None:
            for j in range(ncores):
                nc.sync.dma_start(a2a_out_h[h][j], a2a_in_h[h][j])
        if do_coll in ("both", "ag"):
            nc.gpsimd.collective_compute(
                "AllGather", mybir.AluOpType.bypass, replica_groups=rg,
                ins=[agx_in[h * hrows:(h + 1) * hrows, :]],
                outs=[agx_out_h[h][:]])
        elif do_coll is not None:
            for r in range(ncores):
                nc.sync.dma_start(
                    agx_out_h[h][r * hrows:(r + 1) * hrows, :],
                    agx_in[h * hrows:(h + 1) * hrows, :])

    with tile.TileContext(nc) as tc:
        # ---------------- Phase 1 ----------------
        with (
            tc.tile_pool(name="pin", bufs=4) as pin,
            tc.tile_pool(name="pe_", bufs=3) as pe_,
            tc.tile_pool(name="ppr", bufs=3) as ppr,
            tc.tile_pool(name="pscr", bufs=2) as pscr,
            tc.tile_pool(name="p1s", bufs=8) as p1s,
        ):
            for t in range(row_tiles if stage != 5 else 0):
                h, tl = t // ht, t % ht
                yt = pin.tile([P, C], f32, tag="xt")
                eng_in.dma_start(yt[:], y_in[t * P:(t + 1) * P, :])
                ey = pe_.tile([P, C], f32, tag="et")
                zy = p1s.tile([P, 1], f32, tag="z")
                nc.scalar.activation(ey[:], yt[:], Exp, accum_out=zy[:])
                rzy = p1s.tile([P, 1], f32, tag="rz")
                nc.vector.reciprocal(rzy[:], zy[:])
                pyt = ppr.tile([P, C], bf16, tag="pt")
                nc.vector.tensor_scalar_mul(pyt[:], ey[:], rzy[:])
                eng_out.dma_start(
                    a2a_in_v[h][tl],
                    pyt[:].rearrange("p (j w) -> p j w", j=ncores))

                xt = pin.tile([P, C], f32, tag="xt")
                eng_in.dma_start(xt[:], x_in[t * P:(t + 1) * P, :])
                ex = pe_.tile([P, C], f32, tag="et")
                zx = p1s.tile([P, 1], f32, tag="z")
                nc.scalar.activation(ex[:], xt[:], Exp, accum_out=zx[:])
                eng_tiny.dma_start(zx_out[t], zx[:])
                rzx = p1s.tile([P, 1], f32, tag="rz")
                nc.vector.reciprocal(rzx[:], zx[:])
                pxt = ppr.tile([P, C], bf16, tag="pt")
                nc.vector.tensor_scalar_mul(pxt[:], ex[:], rzx[:])
                eng_out.dma_start(agx_in[t * P:(t + 1) * P, :], pxt[:])
                scr = pscr.tile([P, C], f32, tag="scr")
                dx = p1s.tile([P, 1], f32, tag="dx")
                nc.vector.scalar_tensor_tensor(
                    out=scr[:], in0=ex[:], scalar=1.0, in1=xt[:],
                    op0=mult, op1=mult, accum_out=dx[:])
                eng_out.dma_start(dx_out[t], dx[:])
                if tl == ht - 1:
                    emit_collectives(h)

            if stage == 5:
                emit_collectives(0)
                emit_collectives(1)

        if stage < 3:
            nc.compile()
            return nc

        # ---------------- Phase 3 ----------------
        # half-h views with local k index q = r*(kh//ncores) + kt
        ktr = kh // ncores  # k-tiles per rank per half
        rhs_view_h = [a2a_out_h[h][:].rearrange("j (kt p) w -> p (j kt) w", p=P)
                      for h in range(2)]
        lhs_view_h = [agx_out_h[h][:].rearrange("(r kt p) c -> p (r kt) c",
                                                p=P, kt=ktr)
                      for h in range(2)]

        with (
            tc.tile_pool(name="rhsp", bufs=1) as rhsp,
            tc.tile_pool(name="constp3", bufs=1) as constp3,
            tc.tile_pool(name="slabp", bufs=3) as slabp,
            tc.tile_pool(name="jpsum", bufs=2 * mg, space="PSUM") as jpsum,
            tc.tile_pool(name="drain", bufs=3) as drainp,
            tc.tile_pool(name="accp", bufs=4) as accp,
        ):
            rhs = rhsp.tile([P, 2 * kh, W], bf16)
            rhs_chunk = max(1, 4096 // W)
            for h in range(2):
                for kc in range(0, kh, rhs_chunk):
                    ke = min(kh, kc + rhs_chunk)
                    nc.sync.dma_start(rhs[:, h * kh + kc:h * kh + ke, :],
                                      rhs_view_h[h][:, kc:ke, :])

            ones3 = constp3.tile([P, 1], bf16)
            nc.vector.memset(ones3[:], 1.0)
            ln_bias = constp3.tile([P, 1], f32)
            nc.vector.memset(ln_bias[:], float(n_total) * EPS)

            acc_t_prev = None
            psums_of = {}

            def emit_group_half(g, h):
                if h == 0:
                    psums_of[g] = [
                        jpsum.tile([P, W], f32, tag="jp", name=f"jp_{g}_{m}")
                        for m in range(mg)]
                psums = psums_of[g]
                for kc in range(0, kh, k_chunk):
                    slab = slabp.tile([P, k_chunk, mg * P], bf16, tag="slab")
                    nc.sync.dma_start(
                        slab[:],
                        lhs_view_h[h][:, kc:kc + k_chunk,
                                      g * mg * P:(g + 1) * mg * P])
                    for kk in range(k_chunk):
                        k = kc + kk
                        for m in range(mg):
                            nc.tensor.matmul(
                                psums[m][:],
                                slab[:, kk, m * P:(m + 1) * P],
                                rhs[:, h * kh + k, :],
                                start=(h == 0 and k == 0),
                                stop=(h == 1 and k == kh - 1))

            def emit_drain(g):
                nonlocal acc_t_prev
                psums = psums_of.pop(g)
                for m in range(mg):
                    lnt = drainp.tile([P, W], f32, tag="lnt")
                    nc.scalar.activation(lnt[:], psums[m][:], Ln,
                                         bias=ln_bias[:])
                    scr1 = drainp.tile([P, W], f32, tag="scr1")
                    tt_tmp = accp.tile([P, 1], f32, tag="tttmp")
                    nc.vector.scalar_tensor_tensor(
                        out=scr1[:], in0=psums[m][:], scalar=1.0,
                        in1=lnt[:], op0=mult, op1=mult, accum_out=tt_tmp[:])
                    acc_t = accp.tile([P, 1], f32, tag="acct")
                    if acc_t_prev is None:
                        nc.vector.tensor_copy(acc_t[:], tt_tmp[:])
                    else:
                        nc.vector.tensor_add(acc_t[:], acc_t_prev[:], tt_tmp[:])
                    acc_t_prev = acc_t
                    scr2 = drainp.tile([P, W], f32, tag="scr2")
                    st_m = accp.tile([P, 1], f32, tag="stm")
                    nc.vector.tensor_scalar(
                        out=scr2[:], in0=psums[m][:], scalar1=1.0,
                        scalar2=None, op0=mult, op1=add, accum_out=st_m[:])
                    eng_out.dma_start(margxr_out[g * mg + m], st_m[:])

            # pair 0 interleaves halves to fill the collective window
            emit_group_half(0, 0)
            emit_group_half(1, 0)
            emit_group_half(0, 1)
            emit_group_half(1, 1)
            emit_drain(0)
            emit_drain(1)
            for g in range(2, n_groups):
                emit_group_half(g, 0)
                emit_group_half(g, 1)
                emit_drain(g)

            eng_out.dma_start(tlogt_out[:], acc_t_prev[:])

            # margY (full rhs present by now; psum slot from the group pool)
            psum_my = jpsum.tile([1, W], f32, tag="jp", name="psum_my")
            for k in range(2 * kh):
                nc.tensor.matmul(psum_my[:], ones3[:, 0:1], rhs[:, k, :],
                                 start=(k == 0), stop=(k == 2 * kh - 1))
            margy_sb = constp3.tile([1, W], f32)
            nc.vector.tensor_copy(margy_sb[:], psum_my[:])
            eng_out.dma_start(margy_out[:], margy_sb[:])

    nc.compile()
    return nc


_CACHE = {}


def _get_compiled(key=(N_TOTAL, C_DIM, N_CORES)):
    if key not in _CACHE:
        _CACHE[key] = build_nc(*key)
    return _CACHE[key]


def combine_host(results, n_total=N_TOTAL, C=C_DIM, ncores=N_CORES):
    """Combine per-core partial outputs into the [2] f32 result (fp64 math)."""
    n = float(n_total)
    ent_sum = 0.0
    s_tln = 0.0
    s_t = 0.0
    margx = np.zeros(C, dtype=np.float64)
    margy_blocks = []
    for r in results:
        z = r["zx"].astype(np.float64).ravel()
        d = r["dx"].astype(np.float64).ravel()
        ent_sum += np.sum(np.log(z) - d / z)
        s_tln += float(np.sum(r["tlogt"].astype(np.float64)))
        mxr = r["margxr"].astype(np.float64).reshape(-1)  # [m_tiles*128] = [C]
        s_t += float(mxr.sum())
        margx += mxr
        margy_blocks.append(r["margy"].astype(np.float64).ravel())
    margy = np.concatenate(margy_blocks)
    entropy = ent_sum / n
    S1 = (s_tln - np.log(n) * s_t) / n
    mX = margx / n
    mY = margy / n
    mi = S1 - np.sum(mX * np.log(mX + EPS)) - np.sum(mY * np.log(mY + EPS))
    return np.array([entropy, mi], dtype=np.float32)


def kernel(act_X, act_Y):
    from concourse.bass_utils import run_bass_kernel_spmd

    act_X = np.ascontiguousarray(np.asarray(act_X, dtype=np.float32))
    act_Y = np.ascontiguousarray(np.asarray(act_Y, dtype=np.float32))
    assert act_X.shape == (N_TOTAL, C_DIM) and act_Y.shape == (N_TOTAL, C_DIM)

    nc = _get_compiled()
    n_shard = N_TOTAL // N_CORES
    in_maps = [
        {"x": act_X[k * n_shard:(k + 1) * n_shard],
         "y": act_Y[k * n_shard:(k + 1) * n_shard]}
        for k in range(N_CORES)
    ]
    res = run_bass_kernel_spmd(nc, in_maps, list(range(N_CORES)))
    return combine_host(res.results)

